# revision 21
# baseline (speedup 1.0000x reference)
"""Trainium2 kernel for the Applied-Hamiltonian derivative problem.

Math (see reference):
    H = H0 + H1(t),  H1 = sum_i kron(I, s_i, I) with s_i complex 2x2 per qubit site
    dUr = (H0 + Hr) @ Ui + Hi @ Ur
    dUi = Hi @ Ui - (H0 + Hr) @ Ur

Structure exploited:
  * Hr and Hi are sparse (<= 12 nonzeros/row: a diagonal plus one off-diagonal
    per site at stride 2^k).  Hr is folded into G = H0 + Hr on the host
    (cheap scatter-add), leaving exactly 2 dense 2048^3 GEMMs on the device.
  * Hi's action decomposes per 128-row tile T as
        (Hi @ X)[T] = L_T @ X[T] + sum_{j<4} c_j(T) * X[T ^ e_j]
    where L_T is a 128x128 matrix (low sites + diagonal) and the 4 high
    sites are scalar couplings between row tiles.  L_T rides the dense PSUM
    chain as one extra TensorE matmul (17 instead of 16 per 128x512 output
    tile); the high-site part W = sum_j c_j * X[T^e_j] is combined on the
    otherwise-idle VectorE (4 fused scalar_tensor_tensor ops per chain) and
    fused into the PSUM->SBUF epilogue, off the TensorE critical path.
  * Shipping Urneg = -Ur lets both output planes come straight out of PSUM
    with no epilogue negation.

Schedule (the MM stream runs at the N=512 issue roofline ~216ns/MM with zero
gaps, so all wins are at the edges; measured ~74.9us vs the 94.3us baseline):
  * All input DMAs go on the sync HWDGE queue in consumption order (a single
    HW queue drains FIFO, so k-tile i completes before k-tile i+1); single
    k-tile chunks up front so the PE's k-consumption (1.73us/tile) never
    outruns delivery (~1.0us/tile + ~0.3us/DMA overhead).
  * k-tile 0 splits into [gtA|ui] + [urn] so the plane-0 working set lands
    ~0.5us earlier (the stream's first ~1MB moves at only ~200GB/s — an
    HBM-side ramp); kt0's matmuls run plane-major to match.
  * 13 N=256 warm-up matmuls on scratch fill the PE pipeline from
    preamble-end (~7.8us) until the first chunk lands (~10.4us), releasing
    the HAM clock-gate right as real work starts.
  * Wave A (row-tiles 0-3 x 2 planes, all 8 PSUM banks) is k-major so each
    arriving k-tile feeds all 8 chains.  Its PSUM banks are released by
    ACT-copies on the otherwise-idle ScalarE so wave B starts with zero PE
    bubble (a >2us bubble would also re-throttle the HAM clock-gate); the
    W-add then runs on the DVE off the release path.
  * Wave B's data is fully resident, so it runs pair-major: each row-tile's
    2 chains finish every ~7.3us and their fused epilogue STT + output DMA
    overlap the next pair's matmuls.
  * Outputs are written bf16 (error budget has ~4x slack vs the 2e-2 gate),
    halving output DMA bytes; the final pair staggers its two chain ends and
    splits its two DMAs across the sync+scalar HWDGE queues so the tail is
    just STT + one 128KB DMA receipt + the exit barrier (~4.9us).

Sharding: 2 row-groups x 4 col-groups over 8 cores.  Each core computes
out[p*1024:(p+1)*1024, q*512:(q+1)*512] for both planes.  To keep the SPMD
graph identical across cores, the K row-tiles of gt/ui/urn are XOR-permuted
by 8*p on the host so tile-partner indices are core-independent.

Compute dtype bf16 (inputs pre-cast on host), accumulation fp32 in PSUM.
"""

import numpy as np
import ml_dtypes

import concourse.bass as bass
import concourse.mybir as mybir
import concourse.tile as tile
from concourse.bass_utils import run_bass_kernel_spmd

T_TOTAL = 10.0
N_SITES = 11
DIM = 2048
P = 128
NT = DIM // P          # 16 row/k tiles of the full problem
PR, PC = 2, 4          # row groups x col groups = 8 cores
ROWS = DIM // PR       # 1024 output rows per core
COLS = DIM // PC       # 512 output cols per core
LT = ROWS // P         # 8 output row-tiles per core
BF16 = mybir.dt.bfloat16
F32 = mybir.dt.float32
BF = ml_dtypes.bfloat16
MUL = mybir.AluOpType.mult
ADD = mybir.AluOpType.add

_NC_CACHE = None
_RUN_KWARGS = {}    # test harness can inject trace=True etc.
_LAST_RESULT = None  # BassKernelResults of the most recent run


def _build_graph():
    nc = bass.Bass()
    # gt/ui/urn are shipped already in SBUF layout [128, NT, *] so every DMA
    # is one contiguous descriptor per partition (fast HWDGE issue).
    gu_ext = nc.declare_dram_parameter(
        "gu", [P, NT, ROWS + 2 * COLS], BF16, isOutput=False)
    lci_ext = nc.declare_dram_parameter("lci", [P, 2, LT, P], BF16, isOutput=False)
    # per-(sign, tile, site) high-site coefficients for the DVE combinations
    ch_ext = nc.declare_dram_parameter("ch", [P, 2, LT, 4], F32, isOutput=False)
    out_ext = nc.declare_dram_parameter("out", [2, ROWS, COLS], BF16, isOutput=True)

    out_tv = out_ext[:].rearrange("s (tl p) n -> s tl p n", p=P)

    with tile.TileContext(nc) as tc:
        with (
            tc.tile_pool(name="big", bufs=1) as big,
            tc.tile_pool(name="outp", bufs=12) as outp,
            tc.tile_pool(name="wp", bufs=16) as wpool,
            tc.tile_pool(name="tp", bufs=2) as tpool,
            tc.tile_pool(name="psum", bufs=8, space="PSUM") as psump,
        ):
            gu_sb = big.tile([P, NT, ROWS + 2 * COLS], BF16, tag="gu")
            # [gtA | ui | urn | gtB]: wave A's weights travel with ui/urn so
            # the ramp's critical DMA bytes per k-tile shrink by 25%; the gtB
            # halves ship afterwards (wave B starts much later).
            ui_sb = gu_sb[:, :, 512:1024]
            urn_sb = gu_sb[:, :, 1024:1536]

            def gt_lhsT(kt, tl):
                off = tl * P if tl < 4 else 1536 + (tl - 4) * P
                return gu_sb[:, kt, off:off + P]
            lci_sb = big.tile([P, 2, LT, P], BF16, tag="lci")
            ch_sb = big.tile([P, 2, LT, 4], F32, tag="ch")

            # All input chunks on the sync HWDGE queue, in the exact order
            # the PE consumes them; the single HW queue drains FIFO so early
            # k-tiles complete first.  (The early stream runs at only ~200
            # GB/s — an HBM-side ramp shared by both HWDGE queues, so
            # splitting k0 across queues does not help; measured.)  k-tile 0
            # splits into [gtA|ui] + [urn] on the same queue: the plane-0
            # working set lands ~0.45us earlier and kt0 runs plane-major.
            # Singles through k4 track the PE's 1.73us/k-tile consumption;
            # ch rides mid-stream (needed by the W x^8 terms from ~30us),
            # lci just before gtB (L-matmuls ~39us).
            nc.sync.dma_start(gu_sb[:, 0:1, 0:1024], gu_ext[:, 0:1, 0:1024])
            nc.sync.dma_start(gu_sb[:, 0:1, 1024:1536],
                              gu_ext[:, 0:1, 1024:1536])
            for lo, hi in ((1, 2), (2, 3), (3, 4), (4, 5), (5, 7), (7, 9)):
                nc.sync.dma_start(gu_sb[:, lo:hi, 0:1536],
                                  gu_ext[:, lo:hi, 0:1536])
            nc.sync.dma_start(ch_sb[:], ch_ext[:])
            for lo, hi in ((9, 12), (12, 16)):
                nc.sync.dma_start(gu_sb[:, lo:hi, 0:1536],
                                  gu_ext[:, lo:hi, 0:1536])
            nc.sync.dma_start(lci_sb[:], lci_ext[:])
            for lo, hi in ((0, 8), (8, 16)):
                nc.sync.dma_start(gu_sb[:, lo:hi, 1536:2048],
                                  gu_ext[:, lo:hi, 1536:2048])

            # HAM warm-up: the PE clock-gate needs ~3.4us of sustained matmul
            # activity to reach 2.4 GHz, and the first k-tile takes ~3.8us to
            # land after the preamble.  Burn that window on dummy matmuls
            # over scratch so real matmuls issue at full rate on arrival;
            # N=256 at the cold clock is ~213ns/matmul, so 14 of them bridge
            # the gap with fine granularity.
            # one scratch tile serves as both operands (lhsT = its first 128
            # cols) so a single memset gates the first warm-up matmul
            warm = tpool.tile([P, 256], BF16, tag="wr", name="warm")
            nc.vector.memset(warm[:], 0.0)
            warm_ps = psump.tile([P, 256], F32, tag="ps", name="warm_ps")
            for wi in range(13):
                nc.tensor.matmul(warm_ps[:], warm[:, 0:P], warm[:],
                                 start=(wi == 0), stop=(wi == 12))

            # High-site combinations on DVE:
            #   W[tl,s] = sum_j ch[j] * src[tl^e_j],  e = (8,4,2,1) for j=0..3
            # one tensor_scalar_mul + three fused scalar_tensor_tensor ops per
            # chain, accumulated in place.  Emission is grouped by TERM, in
            # k-tile-arrival order, so the in-order DVE never head-of-line
            # blocks on a late chunk while earlier-ready work exists.
            wt = {}

            def w_ops(tls, term_order):
                for xor, j in term_order:
                    for tl in tls:
                        for s in (0, 1):
                            src = urn_sb if s == 0 else ui_sb
                            if (tl, s) not in wt:
                                w = wpool.tile([P, COLS], BF16, tag="w",
                                               name=f"w_{tl}_{s}")
                                wt[tl, s] = w
                                nc.vector.tensor_scalar_mul(
                                    w[:], src[:, tl ^ xor],
                                    ch_sb[:, s, tl, j:j + 1])
                            else:
                                w = wt[tl, s]
                                nc.vector.scalar_tensor_tensor(
                                    w[:], src[:, tl ^ xor],
                                    ch_sb[:, s, tl, j:j + 1], w[:],
                                    op0=MUL, op1=ADD)

            # wave-A chains touch tiles {tl^1,tl^2}<4 first, then 4-7, then
            # 8-11; wave-B chains touch {tl^4}<4 first, then 4-7, then 12-15.
            # Wave B's first three term groups are emitted here too, filling
            # the DVE's idle window during wave A's matmuls.
            w_ops(range(0, 4), ((1, 3), (2, 2), (4, 1), (8, 0)))
            w_ops(range(4, LT), ((4, 1), (1, 3), (2, 2)))

            def epilogue(tl, s, ps, dma_engine):
                # og = W + psum, straight to bf16; one DVE op releases the
                # PSUM bank and feeds the output DMA.
                og = outp.tile([P, COLS], BF16, tag="og", name=f"og_{tl}_{s}")
                nc.vector.scalar_tensor_tensor(
                    og[:], wt[tl, s][:], 1.0, ps[:], op0=MUL, op1=ADD)
                dma_engine.dma_start(out_tv[s, tl], og[:])

            # Wave A: 8 PSUM chains (4 row-tiles x 2 planes), k-major so the
            # PE consumes k-tiles as the DMAs land.  The PSUM banks must hand
            # over to wave B with zero PE bubble (a >2us bubble also trips the
            # HAM clock-gate), so release goes through the otherwise-idle
            # ScalarE: ACT-copy to bf16 frees the bank ~0.7us after each
            # chain's L-matmul; the W-add runs later on the DVE, in place.
            wave = range(0, 4)
            ps = {}
            for tl in wave:
                for s in (0, 1):
                    ps[tl, s] = psump.tile([P, COLS], F32, tag="ps",
                                           name=f"ps_{tl}_{s}")
            # kt0 runs plane-major so its four plane-0 matmuls (fed by the
            # leading gtA|ui chunk) run while urn(k0) is still in flight.
            for s in (0, 1):
                rhs = ui_sb if s == 0 else urn_sb
                for tl in wave:
                    nc.tensor.matmul(ps[tl, s][:], gt_lhsT(0, tl), rhs[:, 0],
                                     start=True, stop=False)
            for kt in range(1, NT):
                for tl in wave:
                    lhsT = gt_lhsT(kt, tl)
                    nc.tensor.matmul(ps[tl, 0][:], lhsT, ui_sb[:, kt],
                                     start=False, stop=False)
                    nc.tensor.matmul(ps[tl, 1][:], lhsT, urn_sb[:, kt],
                                     start=False, stop=False)
            oga = {}
            for tl in wave:
                for s in (0, 1):
                    other = urn_sb if s == 0 else ui_sb
                    nc.tensor.matmul(ps[tl, s][:], lci_sb[:, s, tl],
                                     other[:, tl], start=False, stop=True)
                for s in (0, 1):
                    og = outp.tile([P, COLS], BF16, tag="og",
                                   name=f"og_{tl}_{s}")
                    nc.scalar.copy(og[:], ps[tl, s][:])
                    oga[tl, s] = og
            for tl in wave:
                for s in (0, 1):
                    og = oga[tl, s]
                    nc.vector.scalar_tensor_tensor(
                        og[:], wt[tl, s][:], 1.0, og[:], op0=MUL, op1=ADD)
                    nc.sync.dma_start(out_tv[s, tl], og[:])

            w_ops(range(4, LT), ((8, 0),))

            # Wave B: data fully resident, so run pair-major — each row-tile's
            # two chains complete every ~7.3us and their epilogues overlap the
            # next pair's matmuls.  The final pair staggers its two chain ends
            # so the s=0 epilogue overlaps the s=1 tail, and its two output
            # DMAs go on different HWDGE queues.
            for tl in range(4, LT):
                ps0 = psump.tile([P, COLS], F32, tag="ps", name=f"ps_{tl}_0")
                ps1 = psump.tile([P, COLS], F32, tag="ps", name=f"ps_{tl}_1")
                last = tl == LT - 1
                for kt in range(NT):
                    lhsT = gt_lhsT(kt, tl)
                    nc.tensor.matmul(ps0[:], lhsT, ui_sb[:, kt],
                                     start=(kt == 0), stop=False)
                    if not (last and kt == NT - 1):
                        nc.tensor.matmul(ps1[:], lhsT, urn_sb[:, kt],
                                         start=(kt == 0), stop=False)
                nc.tensor.matmul(ps0[:], lci_sb[:, 0, tl], urn_sb[:, tl],
                                 start=False, stop=True)
                if last:
                    epilogue(tl, 0, ps0, nc.sync)
                    nc.tensor.matmul(ps1[:], gt_lhsT(NT - 1, tl),
                                     urn_sb[:, NT - 1], start=False, stop=False)
                nc.tensor.matmul(ps1[:], lci_sb[:, 1, tl], ui_sb[:, tl],
                                 start=False, stop=True)
                if not last:
                    epilogue(tl, 0, ps0, nc.scalar)
                epilogue(tl, 1, ps1, nc.scalar)
    return nc


def _split_sync_waits(nc, cap=1):
    """Walrus's per-instruction sync-wait slots are limited (DMA DIRECT2D
    rejects 2, the final drain's 14 are far over).  Engines execute their
    stream serially, so hoisting excess waits into preceding NoOps on the
    same engine is semantically identical."""
    for fn in nc.m.functions:
        for bb in fn.blocks:
            new_insts = []
            for inst in bb.instructions:
                si = getattr(inst, "sync_info", None)
                waits = list(si.on_wait) if si is not None and si.on_wait else []
                if len(waits) > cap:
                    extra, keep = waits[:-cap], waits[-cap:]
                    for i in range(0, len(extra), cap):
                        new_insts.append(mybir.InstNoOp(
                            name=f"{inst.name}-wsplit{i}",
                            engine=inst.engine,
                            bass_nofuse=True,
                            sync_info=mybir.SyncInfo(
                                on_wait=extra[i:i + cap], on_update=[]),
                        ))
                    si.on_wait = keep
                new_insts.append(inst)
            bb.instructions[:] = new_insts


def _get_nc():
    global _NC_CACHE
    if _NC_CACHE is None:
        nc = _build_graph()
        _split_sync_waits(nc)
        _NC_CACHE = nc
    return _NC_CACHE


def _site_ops(A, gates_re, gates_im, t):
    M, NG = A.shape
    n_gates = gates_re.shape[0]
    nsites = NG // n_gates
    a = 0.5 * (T_TOTAL / M)
    tm = np.arange(M, dtype=np.float64) * (T_TOTAL / M)
    env = np.exp(-np.square(float(t) - tm) / (a * a))
    coef = (env @ A.astype(np.float64)).reshape(n_gates, nsites)
    site_re = np.einsum("gn,gab->nab", coef, gates_re.astype(np.float64))
    site_im = np.einsum("gn,gab->nab", coef, gates_im.astype(np.float64))
    return site_re, site_im


def kernel(A, gates_re, gates_im, H0, U, t):
    A = np.asarray(A)
    gates_re = np.asarray(gates_re)
    gates_im = np.asarray(gates_im)
    H0 = np.asarray(H0)
    U = np.asarray(U)
    t = float(np.asarray(t))

    site_re, site_im = _site_ops(A, gates_re, gates_im, t)
    nsites = N_SITES
    strides = [2 ** (nsites - 1 - i) for i in range(nsites)]
    r = np.arange(DIM)
    bits = [((r >> (nsites - 1 - i)) & 1) for i in range(nsites)]

    # G = H0 + Hr via scatter-add (Hr has <= 12 nonzeros per row)
    G = H0.astype(np.float32).copy()
    diag = np.zeros(DIM)
    for i in range(nsites):
        diag += site_re[i][bits[i], bits[i]]
    G[r, r] += diag.astype(np.float32)
    for i in range(nsites):
        G[r, r ^ strides[i]] += site_re[i][bits[i], 1 - bits[i]].astype(np.float32)

    # Per-tile low-site operators and high-site couplings of Hi
    p = np.arange(P)
    L = np.zeros((NT, P, P))
    chigh = np.zeros((NT, 4))
    dlow = np.zeros(P)
    for i in range(4, nsites):
        bp = (p >> (nsites - 1 - i)) & 1
        dlow += site_im[i][bp, bp]
    Loff = np.zeros((P, P))
    for i in range(4, nsites):
        bp = (p >> (nsites - 1 - i)) & 1
        Loff[p, p ^ strides[i]] += site_im[i][bp, 1 - bp]
    for T in range(NT):
        d_high = 0.0
        for i in range(4):
            bT = (T >> (3 - i)) & 1
            d_high += site_im[i][bT, bT]
            chigh[T, i] = site_im[i][bT, 1 - bT]
        Lmat = Loff.copy()
        Lmat[p, p] += d_high + dlow
        L[T] = Lmat

    Ur, Ui = U[0], U[1]
    in_maps = []
    for core in range(8):
        pg, qg = divmod(core, PC)
        tile_order = [s ^ (LT * pg) for s in range(NT)]
        rows = slice(pg * ROWS, (pg + 1) * ROWS)
        cols = slice(qg * COLS, (qg + 1) * COLS)

        # SBUF layout [p, kt, gt|ui|urn]: partition-major, packed so each
        # k-chunk loads with a single contiguous DMA
        gu_h = np.empty((P, NT, ROWS + 2 * COLS), BF)
        gt_full = (
            G[rows, :].T.reshape(NT, P, ROWS)[tile_order].transpose(1, 0, 2)
        ).astype(BF)
        gu_h[:, :, 0:512] = gt_full[:, :, 0:512]          # gtA (tl 0-3)
        gu_h[:, :, 1536:2048] = gt_full[:, :, 512:1024]   # gtB (tl 4-7)
        gu_h[:, :, 512:1024] = (
            Ui[:, cols].reshape(NT, P, COLS)[tile_order].transpose(1, 0, 2)
        ).astype(BF)
        gu_h[:, :, 1024:1536] = (
            (-Ur[:, cols]).reshape(NT, P, COLS)[tile_order].transpose(1, 0, 2)
        ).astype(BF)

        # lci[k, s, tl, m] = sign_s * L[tg][m, k]   (lhsT layout)
        tgs = [(LT * pg) ^ tl for tl in range(LT)]
        lci_h = np.empty((P, 2, LT, P), np.float64)
        ch_h = np.empty((P, 2, LT, 4), np.float32)
        for tl in range(LT):
            lci_h[:, 0, tl] = -L[tgs[tl]].T
            lci_h[:, 1, tl] = L[tgs[tl]].T
            for j in range(4):
                c = np.float32(chigh[tgs[tl], j])
                ch_h[:, 0, tl, j] = -c
                ch_h[:, 1, tl, j] = c
        in_maps.append({
            "gu": gu_h,
            "lci": lci_h.astype(BF),
            "ch": ch_h,
        })

    nc = _get_nc()
    res = run_bass_kernel_spmd(nc, in_maps, core_ids=list(range(8)), **_RUN_KWARGS)
    global _LAST_RESULT
    _LAST_RESULT = res
    out = np.empty((2, DIM, DIM), np.float32)
    for core in range(8):
        pg, qg = divmod(core, PC)
        out[:, pg * ROWS:(pg + 1) * ROWS, qg * COLS:(qg + 1) * COLS] = (
            res.results[core]["out"].astype(np.float32)
        )
    return out


# revision 24
# speedup vs baseline: 1.0113x; 1.0113x over previous
"""Trainium2 kernel for the Applied-Hamiltonian derivative problem.

Math (see reference):
    H = H0 + H1(t),  H1 = sum_i kron(I, s_i, I) with s_i complex 2x2 per qubit site
    dUr = (H0 + Hr) @ Ui + Hi @ Ur
    dUi = Hi @ Ui - (H0 + Hr) @ Ur

Structure exploited:
  * Hr and Hi are sparse (<= 12 nonzeros/row: a diagonal plus one off-diagonal
    per site at stride 2^k).  Hr is folded into G = H0 + Hr on the host
    (cheap scatter-add), leaving exactly 2 dense 2048^3 GEMMs on the device.
  * Hi's action decomposes per 128-row tile T as
        (Hi @ X)[T] = L_T @ X[T] + sum_{j<4} c_j(T) * X[T ^ e_j]
    where L_T is a 128x128 matrix (low sites + diagonal) and the 4 high
    sites are scalar couplings between row tiles.  L_T rides the dense PSUM
    chain as one extra TensorE matmul (17 instead of 16 per 128x512 output
    tile); the high-site part W = sum_j c_j * X[T^e_j] is combined on the
    otherwise-idle VectorE (4 fused scalar_tensor_tensor ops per chain) and
    fused into the PSUM->SBUF epilogue, off the TensorE critical path.
  * Shipping Urneg = -Ur lets both output planes come straight out of PSUM
    with no epilogue negation.

Schedule (the MM stream runs at the N=512 issue roofline ~216ns/MM with zero
gaps, so all wins are at the edges; measured ~74.9us vs the 94.3us baseline):
  * All input DMAs go on the sync HWDGE queue in consumption order (a single
    HW queue drains FIFO, so k-tile i completes before k-tile i+1); single
    k-tile chunks up front so the PE's k-consumption (1.73us/tile) never
    outruns delivery (~1.0us/tile + ~0.3us/DMA overhead).
  * k-tile 0 splits into [gtA|ui] + [urn] so the plane-0 working set lands
    ~0.5us earlier (the stream's first ~1MB moves at only ~200GB/s — an
    HBM-side ramp); kt0's matmuls run plane-major to match.
  * 13 N=256 warm-up matmuls on scratch fill the PE pipeline from
    preamble-end (~7.8us) until the first chunk lands (~10.4us), releasing
    the HAM clock-gate right as real work starts.
  * Wave A (row-tiles 0-3 x 2 planes, all 8 PSUM banks) is k-major so each
    arriving k-tile feeds all 8 chains.  Its PSUM banks are released by
    ACT-copies on the otherwise-idle ScalarE so wave B starts with zero PE
    bubble (a >2us bubble would also re-throttle the HAM clock-gate); the
    W-add then runs on the DVE off the release path.
  * Wave B's data is fully resident, so it runs pair-major: each row-tile's
    2 chains finish every ~7.3us and their fused epilogue STT + output DMA
    overlap the next pair's matmuls.
  * Outputs are written bf16 (error budget has ~4x slack vs the 2e-2 gate),
    halving output DMA bytes; the final pair staggers its two chain ends and
    splits its two DMAs across the sync+scalar HWDGE queues so the tail is
    just STT + one 128KB DMA receipt + the exit barrier (~4.9us).

Sharding: 2 row-groups x 4 col-groups over 8 cores.  Each core computes
out[p*1024:(p+1)*1024, q*512:(q+1)*512] for both planes.  To keep the SPMD
graph identical across cores, the K row-tiles of gt/ui/urn are XOR-permuted
by 8*p on the host so tile-partner indices are core-independent.

Compute dtype bf16 (inputs pre-cast on host), accumulation fp32 in PSUM.
"""

import numpy as np
import ml_dtypes

import concourse.bass as bass
import concourse.mybir as mybir
import concourse.tile as tile
from concourse.bass_utils import run_bass_kernel_spmd

T_TOTAL = 10.0
N_SITES = 11
DIM = 2048
P = 128
NT = DIM // P          # 16 row/k tiles of the full problem
PR, PC = 2, 4          # row groups x col groups = 8 cores
ROWS = DIM // PR       # 1024 output rows per core
COLS = DIM // PC       # 512 output cols per core
LT = ROWS // P         # 8 output row-tiles per core
BF16 = mybir.dt.bfloat16
F32 = mybir.dt.float32
BF = ml_dtypes.bfloat16
MUL = mybir.AluOpType.mult
ADD = mybir.AluOpType.add

_NC_CACHE = None
_RUN_KWARGS = {}    # test harness can inject trace=True etc.
_LAST_RESULT = None  # BassKernelResults of the most recent run


def _build_graph():
    nc = bass.Bass()
    # gt/ui/urn are shipped already in SBUF layout [128, NT, *] so every DMA
    # is one contiguous descriptor per partition (fast HWDGE issue).
    gu_ext = nc.declare_dram_parameter(
        "gu", [P, NT, ROWS + 2 * COLS], BF16, isOutput=False)
    lci_ext = nc.declare_dram_parameter("lci", [P, 2, LT, P], BF16, isOutput=False)
    # per-(sign, tile, site) high-site coefficients for the DVE combinations
    ch_ext = nc.declare_dram_parameter("ch", [P, 2, LT, 4], F32, isOutput=False)
    out_ext = nc.declare_dram_parameter("out", [2, ROWS, COLS], BF16, isOutput=True)

    out_tv = out_ext[:].rearrange("s (tl p) n -> s tl p n", p=P)

    with tile.TileContext(nc) as tc:
        with (
            tc.tile_pool(name="big", bufs=1) as big,
            tc.tile_pool(name="outp", bufs=12) as outp,
            tc.tile_pool(name="wp", bufs=16) as wpool,
            tc.tile_pool(name="tp", bufs=2) as tpool,
            tc.tile_pool(name="psum", bufs=8, space="PSUM") as psump,
        ):
            gu_sb = big.tile([P, NT, ROWS + 2 * COLS], BF16, tag="gu")
            # [gtA | ui | urn | gtB]: wave A's weights travel with ui/urn so
            # the ramp's critical DMA bytes per k-tile shrink by 25%; the gtB
            # halves ship afterwards (wave B starts much later).
            ui_sb = gu_sb[:, :, 512:1024]
            urn_sb = gu_sb[:, :, 1024:1536]

            def gt_lhsT(kt, tl):
                off = tl * P if tl < 4 else 1536 + (tl - 4) * P
                return gu_sb[:, kt, off:off + P]
            lci_sb = big.tile([P, 2, LT, P], BF16, tag="lci")
            ch_sb = big.tile([P, 2, LT, 4], F32, tag="ch")

            # All input chunks on the sync HWDGE queue, in the exact order
            # the PE consumes them; the single HW queue drains FIFO so early
            # k-tiles complete first.  (The early stream runs at only ~200
            # GB/s — an HBM-side ramp shared by both HWDGE queues, so
            # splitting k0 across queues does not help; measured.)  k-tile 0
            # splits into [gtA|ui] + [urn] on the same queue: the plane-0
            # working set lands ~0.45us earlier and kt0 runs plane-major.
            # Singles through k4 track the PE's 1.73us/k-tile consumption;
            # ch rides mid-stream (needed by the W x^8 terms from ~30us),
            # lci just before gtB (L-matmuls ~39us).
            nc.sync.dma_start(gu_sb[:, 0:1, 0:1024], gu_ext[:, 0:1, 0:1024])
            nc.sync.dma_start(gu_sb[:, 0:1, 1024:1536],
                              gu_ext[:, 0:1, 1024:1536])
            for lo, hi in ((1, 2), (2, 3), (3, 4), (4, 5), (5, 7), (7, 9)):
                nc.sync.dma_start(gu_sb[:, lo:hi, 0:1536],
                                  gu_ext[:, lo:hi, 0:1536])
            nc.sync.dma_start(ch_sb[:], ch_ext[:])
            for lo, hi in ((9, 12), (12, 16)):
                nc.sync.dma_start(gu_sb[:, lo:hi, 0:1536],
                                  gu_ext[:, lo:hi, 0:1536])
            nc.sync.dma_start(lci_sb[:], lci_ext[:])
            for lo, hi in ((0, 8), (8, 16)):
                nc.sync.dma_start(gu_sb[:, lo:hi, 1536:2048],
                                  gu_ext[:, lo:hi, 1536:2048])

            # HAM warm-up: the PE clock-gate needs ~3.4us of sustained matmul
            # activity to reach 2.4 GHz, and the first k-tile takes ~3.8us to
            # land after the preamble.  Burn that window on dummy matmuls
            # over scratch so real matmuls issue at full rate on arrival;
            # N=256 at the cold clock is ~213ns/matmul, so 14 of them bridge
            # the gap with fine granularity.
            # one scratch tile serves as both operands (lhsT = its first 128
            # cols) so a single memset gates the first warm-up matmul
            warm = tpool.tile([P, 256], BF16, tag="wr", name="warm")
            nc.vector.memset(warm[:], 0.0)
            warm_ps = psump.tile([P, 256], F32, tag="ps", name="warm_ps")
            for wi in range(11):
                nc.tensor.matmul(warm_ps[:], warm[:, 0:P], warm[:],
                                 start=(wi == 0), stop=(wi == 10))

            # High-site combinations on DVE:
            #   W[tl,s] = sum_j ch[j] * src[tl^e_j],  e = (8,4,2,1) for j=0..3
            # one tensor_scalar_mul + three fused scalar_tensor_tensor ops per
            # chain, accumulated in place.  Emission is grouped by TERM, in
            # k-tile-arrival order, so the in-order DVE never head-of-line
            # blocks on a late chunk while earlier-ready work exists.
            wt = {}

            def w_ops(tls, term_order):
                for xor, j in term_order:
                    for tl in tls:
                        for s in (0, 1):
                            src = urn_sb if s == 0 else ui_sb
                            if (tl, s) not in wt:
                                w = wpool.tile([P, COLS], BF16, tag="w",
                                               name=f"w_{tl}_{s}")
                                wt[tl, s] = w
                                nc.vector.tensor_scalar_mul(
                                    w[:], src[:, tl ^ xor],
                                    ch_sb[:, s, tl, j:j + 1])
                            else:
                                w = wt[tl, s]
                                nc.vector.scalar_tensor_tensor(
                                    w[:], src[:, tl ^ xor],
                                    ch_sb[:, s, tl, j:j + 1], w[:],
                                    op0=MUL, op1=ADD)

            # wave-A chains touch tiles {tl^1,tl^2}<4 first, then 4-7, then
            # 8-11; wave-B chains touch {tl^4}<4 first, then 4-7, then 12-15.
            # Wave B's first three term groups are emitted here too, filling
            # the DVE's idle window during wave A's matmuls.
            w_ops(range(0, 4), ((1, 3), (2, 2), (4, 1), (8, 0)))
            w_ops(range(4, LT), ((4, 1), (1, 3), (2, 2)))

            def epilogue(tl, s, ps, dma_engine):
                # og = W + psum, straight to bf16; one DVE op releases the
                # PSUM bank and feeds the output DMA.
                og = outp.tile([P, COLS], BF16, tag="og", name=f"og_{tl}_{s}")
                nc.vector.scalar_tensor_tensor(
                    og[:], wt[tl, s][:], 1.0, ps[:], op0=MUL, op1=ADD)
                dma_engine.dma_start(out_tv[s, tl], og[:])

            # Wave A: 8 PSUM chains (4 row-tiles x 2 planes), k-major so the
            # PE consumes k-tiles as the DMAs land.  The PSUM banks must hand
            # over to wave B with zero PE bubble (a >2us bubble also trips the
            # HAM clock-gate), so release goes through the otherwise-idle
            # ScalarE: ACT-copy to bf16 frees the bank ~0.7us after each
            # chain's L-matmul; the W-add runs later on the DVE, in place.
            wave = range(0, 4)
            ps = {}
            for tl in wave:
                for s in (0, 1):
                    ps[tl, s] = psump.tile([P, COLS], F32, tag="ps",
                                           name=f"ps_{tl}_{s}")
            # kt0 runs plane-major so its four plane-0 matmuls (fed by the
            # leading gtA|ui chunk) run while urn(k0) is still in flight.
            for s in (0, 1):
                rhs = ui_sb if s == 0 else urn_sb
                for tl in wave:
                    nc.tensor.matmul(ps[tl, s][:], gt_lhsT(0, tl), rhs[:, 0],
                                     start=True, stop=False)
            for kt in range(1, NT):
                for tl in wave:
                    lhsT = gt_lhsT(kt, tl)
                    nc.tensor.matmul(ps[tl, 0][:], lhsT, ui_sb[:, kt],
                                     start=False, stop=False)
                    nc.tensor.matmul(ps[tl, 1][:], lhsT, urn_sb[:, kt],
                                     start=False, stop=False)
            oga = {}
            for tl in wave:
                for s in (0, 1):
                    other = urn_sb if s == 0 else ui_sb
                    nc.tensor.matmul(ps[tl, s][:], lci_sb[:, s, tl],
                                     other[:, tl], start=False, stop=True)
                for s in (0, 1):
                    og = outp.tile([P, COLS], BF16, tag="og",
                                   name=f"og_{tl}_{s}")
                    nc.scalar.copy(og[:], ps[tl, s][:])
                    oga[tl, s] = og
            for tl in wave:
                for s in (0, 1):
                    og = oga[tl, s]
                    nc.vector.scalar_tensor_tensor(
                        og[:], wt[tl, s][:], 1.0, og[:], op0=MUL, op1=ADD)
                    nc.sync.dma_start(out_tv[s, tl], og[:])

            w_ops(range(4, LT), ((8, 0),))

            # Wave B: data fully resident, so run pair-major — each row-tile's
            # two chains complete every ~7.3us and their epilogues overlap the
            # next pair's matmuls.  The final pair staggers its two chain ends
            # so the s=0 epilogue overlaps the s=1 tail, and its two output
            # DMAs go on different HWDGE queues.
            for tl in range(4, LT):
                ps0 = psump.tile([P, COLS], F32, tag="ps", name=f"ps_{tl}_0")
                ps1 = psump.tile([P, COLS], F32, tag="ps", name=f"ps_{tl}_1")
                last = tl == LT - 1
                for kt in range(NT):
                    lhsT = gt_lhsT(kt, tl)
                    nc.tensor.matmul(ps0[:], lhsT, ui_sb[:, kt],
                                     start=(kt == 0), stop=False)
                    if not (last and kt == NT - 1):
                        nc.tensor.matmul(ps1[:], lhsT, urn_sb[:, kt],
                                         start=(kt == 0), stop=False)
                nc.tensor.matmul(ps0[:], lci_sb[:, 0, tl], urn_sb[:, tl],
                                 start=False, stop=True)
                if last:
                    epilogue(tl, 0, ps0, nc.sync)
                    nc.tensor.matmul(ps1[:], gt_lhsT(NT - 1, tl),
                                     urn_sb[:, NT - 1], start=False, stop=False)
                nc.tensor.matmul(ps1[:], lci_sb[:, 1, tl], ui_sb[:, tl],
                                 start=False, stop=True)
                if not last:
                    epilogue(tl, 0, ps0, nc.scalar)
                epilogue(tl, 1, ps1, nc.scalar)
    return nc


def _hoist_head(nc):
    """Move the first user instructions — the warm-up memset (DVE) and the
    k0a input DMA (SP) — from the user block into the entry block, right
    after the per-engine register initializers and BEFORE the Tile entry
    barrier.  The barrier only orders the GpSimd constant-pool memsets and
    scratch init, which neither instruction touches (the DMA is HWDGE, the
    memset writes an immediate), so each engine can fire them ~1us earlier:
    the DMA doorbell rings at ~6.2us instead of ~7.15us and the PE warm-up
    (gated on the memset's semaphore) starts ~0.7us earlier, pulling the
    HAM clock-gate release forward by the same amount."""
    blocks = nc.m.functions[0].blocks
    b0, b1 = blocks[0], blocks[1]
    last_rm = max(i for i, inst in enumerate(b0.instructions)
                  if type(inst).__name__ == "InstRegisterMove")
    head = b1.instructions[:2]
    names = [type(i).__name__ for i in head]
    assert names == ["InstMemset", "InstDMACopy"], names
    for inst in head:
        si = getattr(inst, "sync_info", None)
        assert not (si is not None and si.on_wait), (inst.name, si)
    del b1.instructions[:2]
    b0.instructions[last_rm + 1:last_rm + 1] = head


def _split_sync_waits(nc, cap=1):
    """Walrus's per-instruction sync-wait slots are limited (DMA DIRECT2D
    rejects 2, the final drain's 14 are far over).  Engines execute their
    stream serially, so hoisting excess waits into preceding NoOps on the
    same engine is semantically identical."""
    for fn in nc.m.functions:
        for bb in fn.blocks:
            new_insts = []
            for inst in bb.instructions:
                si = getattr(inst, "sync_info", None)
                waits = list(si.on_wait) if si is not None and si.on_wait else []
                if len(waits) > cap:
                    extra, keep = waits[:-cap], waits[-cap:]
                    for i in range(0, len(extra), cap):
                        new_insts.append(mybir.InstNoOp(
                            name=f"{inst.name}-wsplit{i}",
                            engine=inst.engine,
                            bass_nofuse=True,
                            sync_info=mybir.SyncInfo(
                                on_wait=extra[i:i + cap], on_update=[]),
                        ))
                    si.on_wait = keep
                new_insts.append(inst)
            bb.instructions[:] = new_insts


def _get_nc():
    global _NC_CACHE
    if _NC_CACHE is None:
        nc = _build_graph()
        _hoist_head(nc)
        _split_sync_waits(nc)
        _NC_CACHE = nc
    return _NC_CACHE


def _site_ops(A, gates_re, gates_im, t):
    M, NG = A.shape
    n_gates = gates_re.shape[0]
    nsites = NG // n_gates
    a = 0.5 * (T_TOTAL / M)
    tm = np.arange(M, dtype=np.float64) * (T_TOTAL / M)
    env = np.exp(-np.square(float(t) - tm) / (a * a))
    coef = (env @ A.astype(np.float64)).reshape(n_gates, nsites)
    site_re = np.einsum("gn,gab->nab", coef, gates_re.astype(np.float64))
    site_im = np.einsum("gn,gab->nab", coef, gates_im.astype(np.float64))
    return site_re, site_im


def kernel(A, gates_re, gates_im, H0, U, t):
    A = np.asarray(A)
    gates_re = np.asarray(gates_re)
    gates_im = np.asarray(gates_im)
    H0 = np.asarray(H0)
    U = np.asarray(U)
    t = float(np.asarray(t))

    site_re, site_im = _site_ops(A, gates_re, gates_im, t)
    nsites = N_SITES
    strides = [2 ** (nsites - 1 - i) for i in range(nsites)]
    r = np.arange(DIM)
    bits = [((r >> (nsites - 1 - i)) & 1) for i in range(nsites)]

    # G = H0 + Hr via scatter-add (Hr has <= 12 nonzeros per row)
    G = H0.astype(np.float32).copy()
    diag = np.zeros(DIM)
    for i in range(nsites):
        diag += site_re[i][bits[i], bits[i]]
    G[r, r] += diag.astype(np.float32)
    for i in range(nsites):
        G[r, r ^ strides[i]] += site_re[i][bits[i], 1 - bits[i]].astype(np.float32)

    # Per-tile low-site operators and high-site couplings of Hi
    p = np.arange(P)
    L = np.zeros((NT, P, P))
    chigh = np.zeros((NT, 4))
    dlow = np.zeros(P)
    for i in range(4, nsites):
        bp = (p >> (nsites - 1 - i)) & 1
        dlow += site_im[i][bp, bp]
    Loff = np.zeros((P, P))
    for i in range(4, nsites):
        bp = (p >> (nsites - 1 - i)) & 1
        Loff[p, p ^ strides[i]] += site_im[i][bp, 1 - bp]
    for T in range(NT):
        d_high = 0.0
        for i in range(4):
            bT = (T >> (3 - i)) & 1
            d_high += site_im[i][bT, bT]
            chigh[T, i] = site_im[i][bT, 1 - bT]
        Lmat = Loff.copy()
        Lmat[p, p] += d_high + dlow
        L[T] = Lmat

    Ur, Ui = U[0], U[1]
    in_maps = []
    for core in range(8):
        pg, qg = divmod(core, PC)
        tile_order = [s ^ (LT * pg) for s in range(NT)]
        rows = slice(pg * ROWS, (pg + 1) * ROWS)
        cols = slice(qg * COLS, (qg + 1) * COLS)

        # SBUF layout [p, kt, gt|ui|urn]: partition-major, packed so each
        # k-chunk loads with a single contiguous DMA
        gu_h = np.empty((P, NT, ROWS + 2 * COLS), BF)
        gt_full = (
            G[rows, :].T.reshape(NT, P, ROWS)[tile_order].transpose(1, 0, 2)
        ).astype(BF)
        gu_h[:, :, 0:512] = gt_full[:, :, 0:512]          # gtA (tl 0-3)
        gu_h[:, :, 1536:2048] = gt_full[:, :, 512:1024]   # gtB (tl 4-7)
        gu_h[:, :, 512:1024] = (
            Ui[:, cols].reshape(NT, P, COLS)[tile_order].transpose(1, 0, 2)
        ).astype(BF)
        gu_h[:, :, 1024:1536] = (
            (-Ur[:, cols]).reshape(NT, P, COLS)[tile_order].transpose(1, 0, 2)
        ).astype(BF)

        # lci[k, s, tl, m] = sign_s * L[tg][m, k]   (lhsT layout)
        tgs = [(LT * pg) ^ tl for tl in range(LT)]
        lci_h = np.empty((P, 2, LT, P), np.float64)
        ch_h = np.empty((P, 2, LT, 4), np.float32)
        for tl in range(LT):
            lci_h[:, 0, tl] = -L[tgs[tl]].T
            lci_h[:, 1, tl] = L[tgs[tl]].T
            for j in range(4):
                c = np.float32(chigh[tgs[tl], j])
                ch_h[:, 0, tl, j] = -c
                ch_h[:, 1, tl, j] = c
        in_maps.append({
            "gu": gu_h,
            "lci": lci_h.astype(BF),
            "ch": ch_h,
        })

    nc = _get_nc()
    res = run_bass_kernel_spmd(nc, in_maps, core_ids=list(range(8)), **_RUN_KWARGS)
    global _LAST_RESULT
    _LAST_RESULT = res
    out = np.empty((2, DIM, DIM), np.float32)
    for core in range(8):
        pg, qg = divmod(core, PC)
        out[:, pg * ROWS:(pg + 1) * ROWS, qg * COLS:(qg + 1) * COLS] = (
            res.results[core]["out"].astype(np.float32)
        )
    return out
